# revision 1
# baseline (speedup 1.0000x reference)
"""Trainium2 Bass kernel for MoEPred: softmax-gated mixture of 32 tiny experts.

  xi[b] = sum_e softmax_e(x@Wg.T) * (W2[e] . gelu(x @ W1[e] + b1[e]) + b2[e])

Sharding: pure data parallel over batch across 8 NeuronCores; weights are
replicated. x is pre-laid-out on the host so each 512-row macro-tile is one
contiguous 1MB DMA landing as xT chunks [feat 128, rows 512] (the contraction
dim on SBUF partitions). All matmuls run in float32r (full-rate fp32 PE mode).

Per 512-row macro-tile s (software-pipelined across 3 steps so the PE never
waits on ACT/DVE results of the same step):
  step s:   hT [512eh, R] = W1flat.T @ xT   (16 f32r matmuls)   [PE]
            haT = gelu(hT + b1)             (fused bias)        [ACT]
            gT [32, R] = Wg @ xT            (4 matmuls)         [PE]
            st[0:32] = exp(gT)                                  [ACT]
  step s+1: o2T [32, R] = W2bd.T @ haT      (4 matmuls)         [PE]
            st[32:64] = (o2T + b2) * st[0:32]                   [DVE]
  step s+2: sums [33, R] = sel.T @ st       (partition reduce)  [PE]
            xiT [1, R] = sums[32] * recip(sums[0])              [DVE]
"""

import os
import sys
from contextlib import ExitStack

import numpy as np

for _p in ("/opt/trn_rl_repo",):
    if _p not in sys.path:
        sys.path.insert(0, _p)

import jax
from jax.experimental.shard_map import shard_map
from jax.sharding import Mesh, NamedSharding, PartitionSpec

import concourse.bacc as bacc
import concourse.bass2jax as b2j
import concourse.tile as tile
from concourse import mybir

N_CORES = 8
BATCH = 262144
D_IN = 512
N_EXPERTS = 32
HID = 16
EH = N_EXPERTS * HID  # 512
B_LOC = BATCH // N_CORES  # 32768
R = 512  # rows per macro-tile
KC = D_IN // 128  # 4 feature chunks
MC = EH // 128  # 4 eh chunks

F32 = mybir.dt.float32
F32R = mybir.dt.float32r
BF16 = mybir.dt.bfloat16
AF = mybir.ActivationFunctionType
ALU = mybir.AluOpType

_NC_CACHE = {}
_RUNNER_CACHE = {}


def build_nc(b_loc=B_LOC, act_fn=None, loop_n=1, level=5, in_dt=None, coltile=False,
             dma_group=8, xq_bufs=2, dma_rings=2):
    """loop_n > 1 wraps the macro loop in a hardware For_i that redoes the
    identical work loop_n times (benchmark amplification above the ~80-100ms
    axon dispatch floor).
    level: 0=dma, 1=+mm1, 2=+gelu, 3=+gating/exp, 4=+mm2/stt, 5=full.
    dma_group: macros per x-load dma_start (alternating sync/scalar HWDGE)."""
    if act_fn is None:
        act_fn = AF.Gelu
    if in_dt is None:
        in_dt = BF16
    eff = 1 if level in (11, 12) else (3 if level == 13 else level)
    assert b_loc % R == 0
    n_macro = b_loc // R
    G = min(dma_group, n_macro)
    assert n_macro % G == 0
    n_group = n_macro // G

    nc = bacc.Bacc("TRN2", target_bir_lowering=False, debug=False, num_devices=N_CORES)

    xTm = nc.dram_tensor("xTm", [n_group * 128, G * KC * R], in_dt, kind="ExternalInput")
    w1t = nc.dram_tensor("w1t", [D_IN, EH], in_dt, kind="ExternalInput")
    wgt = nc.dram_tensor("wgt", [D_IN, N_EXPERTS], in_dt, kind="ExternalInput")
    w2bd = nc.dram_tensor("w2bd", [EH, N_EXPERTS], in_dt, kind="ExternalInput")
    b1c = nc.dram_tensor("b1c", [128, MC], F32, kind="ExternalInput")
    b2c = nc.dram_tensor("b2c", [N_EXPERTS, 1], F32, kind="ExternalInput")
    sel = nc.dram_tensor("sel", [64, 33], in_dt, kind="ExternalInput")
    outT = nc.dram_tensor("outT", [1, b_loc], F32, kind="ExternalOutput")

    with tile.TileContext(nc) as tc, ExitStack() as ctx:
        const = ctx.enter_context(tc.tile_pool(name="const", bufs=1))
        xpool = ctx.enter_context(tc.tile_pool(name="xp", bufs=xq_bufs))
        hapool = ctx.enter_context(tc.tile_pool(name="hap", bufs=2))
        stpool = ctx.enter_context(tc.tile_pool(name="stp", bufs=3))
        xopool = ctx.enter_context(tc.tile_pool(name="xop", bufs=4))
        ps_h = ctx.enter_context(tc.tile_pool(name="ps_h", bufs=2, space="PSUM"))
        ps_g = ctx.enter_context(tc.tile_pool(name="ps_g", bufs=2, space="PSUM"))
        ps_o = ctx.enter_context(tc.tile_pool(name="ps_o", bufs=2, space="PSUM"))
        ps_s = ctx.enter_context(tc.tile_pool(name="ps_s", bufs=2, space="PSUM"))

        # --- replicated constants, loaded once ---
        w1_sb = const.tile([128, KC * EH], in_dt, name="w1_sb")
        wg_sb = const.tile([128, KC * N_EXPERTS], in_dt, name="wg_sb")
        w2_sb = const.tile([128, MC * N_EXPERTS], in_dt, name="w2_sb")
        b1_sb = const.tile([128, MC], F32, name="b1_sb")
        b2_sb = const.tile([N_EXPERTS, 1], F32, name="b2_sb")
        sel_sb = const.tile([64, 33], in_dt, name="sel_sb")
        for k in range(KC):
            nc.sync.dma_start(w1_sb[:, k * EH:(k + 1) * EH], w1t[k * 128:(k + 1) * 128, :])
            nc.sync.dma_start(wg_sb[:, k * 32:(k + 1) * 32], wgt[k * 128:(k + 1) * 128, :])
            nc.sync.dma_start(w2_sb[:, k * 32:(k + 1) * 32], w2bd[k * 128:(k + 1) * 128, :])
        nc.sync.dma_start(b1_sb[:], b1c[:, :])
        nc.sync.dma_start(b2_sb[:], b2c[:, :])
        nc.sync.dma_start(sel_sb[:], sel[:, :])

        if loop_n > 1:
            ctx.enter_context(tc.For_i(0, loop_n, 1))

        # software pipeline state
        pend1 = []  # (j, ha, st): needs MM2 + stt
        pend2 = []  # (j, st): needs sums + div + out dma

        def emit_stage1_tail(gate_mm=None):
            jj, ha_p, st_p = pend1.pop(0)
            if coltile:
                opt = ps_o.tile([64, R], F32, tag="op", name="op")
                op = opt[32:64]
                pos = (0, 32)
            else:
                op = ps_o.tile([N_EXPERTS, R], F32, tag="op", name="op")
                pos = None
            for m in range(MC):
                nc.tensor.matmul(op[:], lhsT=w2_sb[:, m * 32:(m + 1) * 32],
                                 rhs=ha_p[:, m * R:(m + 1) * R],
                                 start=(m == 0), stop=(m == MC - 1),
                                 tile_position=pos, skip_group_check=coltile)
                if gate_mm is not None:
                    gate_mm(m)
            if eff >= 4:
                nc.vector.scalar_tensor_tensor(
                    st_p[32:64, :], op[:], b2_sb[:], st_p[0:32, :], ALU.add, ALU.mult)
            pend2.append((jj, st_p))

        def emit_stage2_tail():
            jj, st_p = pend2.pop(0)
            sp = ps_s.tile([33, R], F32, tag="sp", name="sp")
            nc.tensor.matmul(sp[:], lhsT=sel_sb[:], rhs=st_p[:], start=True, stop=True)
            rc = xopool.tile([1, R], F32, tag="rc", name="rc")
            xo = xopool.tile([1, R], F32, tag="xo", name="xo")
            nc.vector.reciprocal(rc[:], sp[0:1, :])
            nc.vector.tensor_mul(xo[:], sp[32:33, :], rc[:])
            nc.gpsimd.dma_start(outT[0:1, jj * R:(jj + 1) * R], xo[:])

        xconst = None
        if level in (11, 12):
            xconst = const.tile([128, KC * R], in_dt, name="xconst")
            nc.sync.dma_start(xconst[:], xTm[0:128, 0:KC * R])

        # group-DMA prefetch: issue loads PREFETCH groups ahead of consumption so
        # the dma_start sits ahead of dependent ACT work in its ring's FIFO
        PREFETCH = max(0, min(xq_bufs - 2, 2))
        xq_tiles = {}

        def issue_group_dma(grp):
            if level == 11 or grp >= n_group:
                return
            xq_t = xpool.tile([128, G * KC * R], in_dt, tag="xq", name="xq")
            eng = nc.sync if (dma_rings == 1 or grp % 2 == 0) else nc.scalar
            eng.dma_start(xq_t[:], xTm[grp * 128:(grp + 1) * 128, :])
            xq_tiles[grp] = xq_t

        for g0 in range(PREFETCH + 1):
            issue_group_dma(g0)

        for j in range(n_macro):
            if j % G == 0:
                grp = j // G
                issue_group_dma(grp + PREFETCH + 1)
                xq = xq_tiles.pop(grp, None)
            if level in (11, 12):
                xj = xconst
            else:
                xj = xq[:, (j % G) * KC * R:(j % G + 1) * KC * R]

            if eff >= 1:
                # hT = W1flat.T @ xT ; gelu(hT + b1) -> ha
                ha = hapool.tile([128, MC * R], in_dt, tag="ha", name="ha")
                for m in range(MC):
                    hp = ps_h.tile([128, R], F32, tag="hp", name="hp")
                    for k in range(KC):
                        nc.tensor.matmul(
                            hp[:],
                            lhsT=w1_sb[:, k * EH + m * 128: k * EH + (m + 1) * 128],
                            rhs=xj[:, k * R:(k + 1) * R],
                            start=(k == 0), stop=(k == KC - 1))
                    if eff >= 2:
                        nc.scalar.activation(
                            ha[:, m * R:(m + 1) * R], hp[:], act_fn,
                            bias=b1_sb[:, m:m + 1], scale=1.0)

            if eff >= 3:
                # gating logits + exp
                gp = ps_g.tile([N_EXPERTS, R], F32, tag="gp", name="gp")

                def gate_mm(k, _gp=gp, _xj=xj):
                    nc.tensor.matmul(_gp[:], lhsT=wg_sb[:, k * 32:(k + 1) * 32],
                                     rhs=_xj[:, k * R:(k + 1) * R],
                                     start=(k == 0), stop=(k == KC - 1),
                                     skip_group_check=coltile)

                if coltile and eff >= 4 and j >= 1:
                    # interleave gating(j) with MM2(j-1) in distinct col groups
                    emit_stage1_tail(gate_mm=gate_mm)
                else:
                    for k in range(KC):
                        gate_mm(k)
                    if eff >= 4 and j >= 1:
                        emit_stage1_tail()
                if level != 13:
                    st = stpool.tile([64, R], in_dt, tag="st", name="st")
                    nc.scalar.activation(st[0:32, :], gp[:], AF.Exp)
                    pend1.append((j, ha, st))

                if eff >= 5 and j >= 2:
                    emit_stage2_tail()
            else:
                xo = xopool.tile([1, R], F32, tag="xo", name="xo")
                nc.vector.memset(xo[:], 0.0)
                nc.gpsimd.dma_start(outT[0:1, j * R:(j + 1) * R], xo[:])

        # drain the pipeline
        if eff >= 4:
            while pend1:
                emit_stage1_tail()
        if eff >= 5:
            while pend2:
                emit_stage2_tail()

    nc.compile()
    return nc


def prep_weights(Wg, W1, b1, W2, b2, np_dt=np.float32):
    w1t = np.ascontiguousarray(
        np.asarray(W1, dtype=np.float32).transpose(1, 0, 2).reshape(D_IN, EH)).astype(np_dt)
    wgt = np.ascontiguousarray(np.asarray(Wg, dtype=np.float32).T).astype(np_dt)
    w2bd = np.zeros((EH, N_EXPERTS), np.float32)
    W2 = np.asarray(W2, dtype=np.float32)
    for e in range(N_EXPERTS):
        w2bd[e * HID:(e + 1) * HID, e] = W2[e]
    w2bd = w2bd.astype(np_dt)
    b1c = np.ascontiguousarray(
        np.asarray(b1, dtype=np.float32).reshape(EH).reshape(MC, 128).T)
    b2c = np.asarray(b2, dtype=np.float32).reshape(N_EXPERTS, 1)
    selm = np.zeros((64, 33), np_dt)
    selm[0:32, 0] = 1.0
    selm[32:64, 32] = 1.0
    return {"w1t": w1t, "wgt": wgt, "w2bd": w2bd, "b1c": b1c, "b2c": b2c,
            "sel": selm}


def layout_x(xc, np_dt=np.float32, dma_group=8):
    """Core shard [B_LOC, D_IN] -> per-group contiguous transposed layout
    [n_group*128, G*KC*R]: xTm[g*128+p, ((i*KC)+k)*R+c] = xc[(g*G+i)*R+c, k*128+p]."""
    n_macro = xc.shape[0] // R
    G = min(dma_group, n_macro)
    n_group = n_macro // G
    return np.ascontiguousarray(
        xc.reshape(n_group, G, R, KC, 128).transpose(0, 4, 1, 3, 2).reshape(
            n_group * 128, G * KC * R)).astype(np_dt)


class Runner:
    """Reusable SPMD executor: the multi-core path of
    concourse.bass2jax.run_bass_via_pjrt, factored so the jitted callable and
    device-resident inputs can be reused across calls (for benchmarking)."""

    def __init__(self, nc, n_cores=N_CORES):
        b2j.install_neuronx_cc_hook()
        self.nc = nc
        self.n_cores = n_cores
        partition_name = (
            nc.partition_id_tensor.name if nc.partition_id_tensor else None
        )
        in_names, out_names, out_avals, zero_outs = [], [], [], []
        for alloc in nc.m.functions[0].allocations:
            if not isinstance(alloc, mybir.MemoryLocationSet):
                continue
            assert alloc.memorylocations
            name = alloc.memorylocations[0].name
            if alloc.kind == "ExternalInput":
                if name != partition_name:
                    in_names.append(name)
            elif alloc.kind == "ExternalOutput":
                out_names.append(name)
                shape = tuple(alloc.tensor_shape)
                dtype = mybir.dt.np(alloc.dtype)
                out_avals.append(jax.core.ShapedArray(shape, dtype))
                zero_outs.append(np.zeros(shape, dtype))
        self.in_names = list(in_names)
        self.out_names = out_names
        self.zero_outs = zero_outs
        n_params = len(in_names)
        n_outs = len(out_names)
        bind_names = in_names + out_names
        if partition_name is not None:
            bind_names.append(partition_name)

        def _body(*args):
            operands = list(args)
            if partition_name is not None:
                operands.append(b2j.partition_id_tensor())
            outs = b2j._bass_exec_p.bind(
                *operands,
                out_avals=tuple(out_avals),
                in_names=tuple(bind_names),
                out_names=tuple(out_names),
                lowering_input_output_aliases=(),
                sim_require_finite=True,
                sim_require_nnan=True,
                nc=nc,
            )
            return tuple(outs)

        devices = jax.devices()[:n_cores]
        assert len(devices) == n_cores
        self.mesh = Mesh(np.asarray(devices), ("core",))
        in_specs = (PartitionSpec("core"),) * (n_params + n_outs)
        out_specs = (PartitionSpec("core"),) * n_outs
        self.fn = jax.jit(
            shard_map(_body, mesh=self.mesh, in_specs=in_specs,
                      out_specs=out_specs, check_rep=False),
            donate_argnums=tuple(range(n_params, n_params + n_outs)),
            keep_unused=True,
        )
        self.sharding = NamedSharding(self.mesh, PartitionSpec("core"))

    def put_inputs(self, in_maps):
        assert len(in_maps) == self.n_cores
        concat = [
            np.concatenate([np.asarray(m[name]) for m in in_maps], axis=0)
            for name in self.in_names
        ]
        return [jax.device_put(a, self.sharding) for a in concat]

    def fresh_outs(self):
        return [
            jax.device_put(
                np.zeros((self.n_cores * z.shape[0], *z.shape[1:]), z.dtype),
                self.sharding,
            )
            for z in self.zero_outs
        ]

    def run(self, dev_inputs, dev_outs=None):
        if dev_outs is None:
            dev_outs = self.fresh_outs()
        return self.fn(*dev_inputs, *dev_outs)


def get_runner(b_loc=B_LOC):
    if b_loc not in _RUNNER_CACHE:
        if b_loc not in _NC_CACHE:
            _NC_CACHE[b_loc] = build_nc(b_loc)
        _RUNNER_CACHE[b_loc] = Runner(_NC_CACHE[b_loc])
    return _RUNNER_CACHE[b_loc]


def make_in_maps(x, Wg, W1, b1, W2, b2, np_dt=np.float32, dma_group=8):
    x = np.asarray(x, dtype=np.float32)
    consts = prep_weights(Wg, W1, b1, W2, b2, np_dt)
    xs = x.reshape(N_CORES, B_LOC, D_IN)
    in_maps = []
    for i in range(N_CORES):
        m = dict(consts)
        m["xTm"] = layout_x(xs[i], np_dt, dma_group)
        in_maps.append(m)
    return in_maps


def kernel(x, Wg, W1, b1, W2, b2):
    os.environ["BASS_NEVER_TRACE"] = "1"
    import ml_dtypes
    in_maps = make_in_maps(x, Wg, W1, b1, W2, b2, np_dt=ml_dtypes.bfloat16)
    runner = get_runner(B_LOC)
    dev_in = runner.put_inputs(in_maps)
    outs = runner.run(dev_in)
    out_t = np.asarray(outs[0])  # [N_CORES * 1, B_LOC]
    return np.ascontiguousarray(out_t.reshape(BATCH, 1))


if __name__ == "__main__":
    rng = np.random.default_rng(0)
    inputs = {
        "x": rng.standard_normal((BATCH, D_IN), dtype=np.float32),
        "Wg": (rng.standard_normal((N_EXPERTS, D_IN)) * 0.02).astype(np.float32),
        "W1": (rng.standard_normal((N_EXPERTS, D_IN, HID)) * 0.02).astype(np.float32),
        "b1": (rng.standard_normal((N_EXPERTS, HID)) * 0.02).astype(np.float32),
        "W2": (rng.standard_normal((N_EXPERTS, HID)) * 0.02).astype(np.float32),
        "b2": (rng.standard_normal((N_EXPERTS,)) * 0.02).astype(np.float32),
    }
    out = kernel(**inputs)
    print(out.shape, out.dtype, out[:4, 0])



# revision 6
# speedup vs baseline: 1.0675x; 1.0675x over previous
"""Trainium2 Bass kernel for MoEPred: softmax-gated mixture of 32 tiny experts.

  xi[b] = sum_e softmax_e(x@Wg.T) * (W2[e] . gelu(x @ W1[e] + b1[e]) + b2[e])

Sharding: pure data parallel over batch across 8 NeuronCores; weights
replicated. x pre-laid-out on host so each 512-row macro-tile streams as xT
chunks [feat 128, rows 512] (contraction dim on SBUF partitions).

Key structure (per 512-row macro-tile, 64 per core):
  MM1   hT[eh,R] = W1.T @ xT        16 bf16 matmuls            [PE]
  gelu  ha = gelu(hT + b1)          4 ACT instrs (bias fused)  [ACT]
  gate  g -> one shared PSUM bank for 4 macros via col groups  [PE]
  MM2   out2 -> shared bank, 4 macro col groups                [PE]
Per 4-macro group (so the small ops run at full 128-partition width):
  softmax exp WITHOUT the exp table (exp and gelu never share an ACT
  table; swapping costs 1283ns):  t = tanh(g/2) [ACT, same table as gelu]
  exp(g) = (1+t)/(1-t)            [DVE]
  st2 = (out2 + b2) * exp         [DVE]
  num/den via two ones-matmuls -> [4,1024] psum -> SBUF -> DRAM; the final
  xi = num/den division happens on HOST (avoids [1,R] 1-partition DVE ops).
"""

import os
import sys
from contextlib import ExitStack

import numpy as np

for _p in ("/opt/trn_rl_repo",):
    if _p not in sys.path:
        sys.path.insert(0, _p)

import jax
from jax.experimental.shard_map import shard_map
from jax.sharding import Mesh, NamedSharding, PartitionSpec

import concourse.bacc as bacc
import concourse.bass2jax as b2j
import concourse.tile as tile
from concourse import mybir

N_CORES = 8
BATCH = 262144
D_IN = 512
N_EXPERTS = 32
HID = 16
EH = N_EXPERTS * HID  # 512
B_LOC = BATCH // N_CORES  # 32768
R = 512  # rows per macro-tile
KC = D_IN // 128  # 4 feature chunks
MC = EH // 128  # 4 eh chunks

F32 = mybir.dt.float32
BF16 = mybir.dt.bfloat16
FP8 = mybir.dt.float8e4
AF = mybir.ActivationFunctionType
ALU = mybir.AluOpType
DR = mybir.MatmulPerfMode.DoubleRow

_NC_CACHE = {}
_RUNNER_CACHE = {}


def build_nc(b_loc=B_LOC, loop_n=1, level=5, dma_group=8, xq_bufs=3,
             dma_rings=2):
    """level: 0=dma, 1=+mm1, 2=+gelu, 3=+gating/exp, 4=+mm2/stt, 5=full."""
    assert b_loc % (4 * R) == 0
    n_macro = b_loc // R
    n4 = n_macro // 4
    G = min(dma_group, n_macro)
    assert n_macro % G == 0 and G % 4 == 0
    n_group = n_macro // G

    nc = bacc.Bacc("TRN2", target_bir_lowering=False, debug=False,
                   num_devices=N_CORES)

    xTm = nc.dram_tensor("xTm", [n_group * 128, G * KC * R], BF16,
                         kind="ExternalInput")
    w1t = nc.dram_tensor("w1t", [D_IN, EH], BF16, kind="ExternalInput")
    wgt = nc.dram_tensor("wgt", [D_IN, N_EXPERTS], BF16, kind="ExternalInput")
    w2bd = nc.dram_tensor("w2bd", [EH, N_EXPERTS], BF16, kind="ExternalInput")
    b1c = nc.dram_tensor("b1c", [128, MC], F32, kind="ExternalInput")
    b2r = nc.dram_tensor("b2r", [128, 1], F32, kind="ExternalInput")
    selm = nc.dram_tensor("selm", [128, 4], BF16, kind="ExternalInput")
    outT = nc.dram_tensor("outT", [4, n4 * 1024], F32, kind="ExternalOutput")

    with tile.TileContext(nc) as tc, ExitStack() as ctx:
        const = ctx.enter_context(tc.tile_pool(name="const", bufs=1))
        xpool = ctx.enter_context(tc.tile_pool(name="xp", bufs=xq_bufs))
        hapool = ctx.enter_context(tc.tile_pool(name="hap", bufs=2))
        vpool = ctx.enter_context(tc.tile_pool(name="vp", bufs=2))
        cpool = ctx.enter_context(tc.tile_pool(name="cp", bufs=2))
        ps_h = ctx.enter_context(tc.tile_pool(name="ps_h", bufs=2, space="PSUM"))
        ps_g = ctx.enter_context(tc.tile_pool(name="ps_g", bufs=2, space="PSUM"))
        ps_o = ctx.enter_context(tc.tile_pool(name="ps_o", bufs=2, space="PSUM"))
        ps_s = ctx.enter_context(tc.tile_pool(name="ps_s", bufs=1, space="PSUM"))

        # --- replicated constants, loaded once ---
        w1_sb = const.tile([128, KC, EH], BF16, name="w1_sb")
        wg_sb = const.tile([128, KC, N_EXPERTS], BF16, name="wg_sb")
        w2_sb = const.tile([128, MC, N_EXPERTS], BF16, name="w2_sb")
        b1_sb = const.tile([128, MC], F32, name="b1_sb")
        b2_sb = const.tile([128, 1], F32, name="b2_sb")
        sel_sb = const.tile([128, 4], BF16, name="sel_sb")
        for k in range(KC):
            nc.sync.dma_start(w1_sb[:, k, :], w1t[k * 128:(k + 1) * 128, :])
            nc.sync.dma_start(wg_sb[:, k, :], wgt[k * 128:(k + 1) * 128, :])
            nc.sync.dma_start(w2_sb[:, k, :], w2bd[k * 128:(k + 1) * 128, :])
        nc.sync.dma_start(b1_sb[:], b1c[:, :])
        nc.sync.dma_start(b2_sb[:], b2r[:, :])
        nc.sync.dma_start(sel_sb[:], selm[:, :])

        if loop_n > 1:
            ctx.enter_context(tc.For_i(0, loop_n, 1))

        # group-DMA prefetch
        PREFETCH = max(0, min(xq_bufs - 2, 2))
        xq_tiles = {}

        def issue_group_dma(grp):
            if grp >= n_group:
                return
            xq_t = xpool.tile([128, G, KC, R], BF16, tag="xq", name="xq")
            eng = nc.sync if (dma_rings == 1 or grp % 2 == 0) else nc.scalar
            eng.dma_start(xq_t[:], xTm[grp * 128:(grp + 1) * 128, :])
            xq_tiles[grp] = xq_t

        for g0 in range(PREFETCH + 1):
            issue_group_dma(g0)

        gb = ob = None
        for j in range(n_macro):
            j4 = j % 4
            g4 = j // 4
            if j % G == 0:
                grp = j // G
                issue_group_dma(grp + PREFETCH + 1)
                xq = xq_tiles.pop(grp, None)
            xj = xq[:, j % G]

            if level >= 1:
                # MM1 + gelu per m-chunk (bias fused into ACT per-chunk)
                ha = hapool.tile([128, MC, R], BF16, tag="ha", name="ha")
                for m in range(MC):
                    hp = ps_h.tile([128, R], F32, tag="hp", name="hp")
                    for k in range(KC):
                        nc.tensor.matmul(
                            hp[:], lhsT=w1_sb[:, k, m * 128:(m + 1) * 128],
                            rhs=xj[:, k, :],
                            start=(k == 0), stop=(k == KC - 1))
                    if level >= 2:
                        nc.scalar.activation(
                            ha[:, m, :], hp[:], AF.Gelu,
                            bias=b1_sb[:, m:m + 1], scale=1.0)

            if level >= 3:
                # gating for 4 macros shares one PSUM bank via col groups:
                # one start=True clears the bank; later groups overwrite
                # where has_written is clear, accumulate where set.
                if j4 == 0:
                    gb = ps_g.tile([128, R], F32, tag="gb", name="gb")
                gsl = gb[32 * j4:32 * (j4 + 1), :]
                for k in range(KC):
                    nc.tensor.matmul(
                        gsl, lhsT=wg_sb[:, k, :], rhs=xj[:, k, :],
                        start=(k == 0), stop=(k == KC - 1),
                        tile_position=(0, 32 * j4),
                        skip_group_check=True)

            if level >= 4:
                if j4 == 0:
                    ob = ps_o.tile([128, R], F32, tag="ob", name="ob")
                osl = ob[32 * j4:32 * (j4 + 1), :]
                for m in range(MC):
                    nc.tensor.matmul(
                        osl, lhsT=w2_sb[:, m, :], rhs=ha[:, m, :],
                        start=(m == 0), stop=(m == MC - 1),
                        tile_position=(0, 32 * j4),
                        skip_group_check=True)

            if j4 == 3:
                if level >= 3:
                    # t = tanh(g/2); exp(g) = (1+t)/(1-t)  [no table swap]
                    th = vpool.tile([128, R], F32, tag="th", name="th")
                    nc.scalar.activation(th[:], gb[:], AF.Tanh,
                                         bias=0.0, scale=0.5)
                    dd = vpool.tile([128, R], F32, tag="dd", name="dd")
                    nc.vector.tensor_scalar(dd[:], th[:], -1.0, 1.0,
                                            ALU.mult, ALU.add)
                    rr = vpool.tile([128, R], F32, tag="rr", name="rr")
                    nc.vector.reciprocal(rr[:], dd[:])
                    exp4 = vpool.tile([128, R], BF16, tag="e4", name="e4")
                    nc.vector.scalar_tensor_tensor(
                        exp4[:], th[:], 1.0, rr[:], ALU.add, ALU.mult)
                if level >= 4:
                    st2 = vpool.tile([128, R], BF16, tag="st2", name="st2")
                    nc.vector.scalar_tensor_tensor(
                        st2[:], ob[:], b2_sb[:], exp4[:], ALU.add, ALU.mult)
                if level >= 5:
                    sp = ps_s.tile([4, 1024], F32, tag="sp", name="sp")
                    nc.tensor.matmul(sp[:, 0:512], lhsT=sel_sb[:],
                                     rhs=exp4[:], start=True, stop=True,
                                     skip_group_check=True)
                    nc.tensor.matmul(sp[:, 512:1024], lhsT=sel_sb[:],
                                     rhs=st2[:], start=True, stop=True,
                                     skip_group_check=True)
                    cp = cpool.tile([4, 1024], F32, tag="cpo", name="cpo")
                    nc.vector.tensor_scalar_add(cp[:], sp[:], 0.0)
                    nc.gpsimd.dma_start(
                        outT[:, g4 * 1024:(g4 + 1) * 1024], cp[:])
                elif level < 5:
                    cp = cpool.tile([4, 1024], F32, tag="cpo", name="cpo")
                    nc.vector.memset(cp[:], 1.0)
                    nc.gpsimd.dma_start(
                        outT[:, g4 * 1024:(g4 + 1) * 1024], cp[:])

    nc.compile()
    return nc


def prep_weights(Wg, W1, b1, W2, b2):
    import ml_dtypes
    bf = ml_dtypes.bfloat16
    W1f = np.ascontiguousarray(
        np.asarray(W1, np.float32).transpose(1, 0, 2).reshape(D_IN, EH))
    wgt = np.ascontiguousarray(np.asarray(Wg, np.float32).T)
    w2bd = np.zeros((EH, N_EXPERTS), np.float32)
    W2 = np.asarray(W2, np.float32)
    for e in range(N_EXPERTS):
        w2bd[e * HID:(e + 1) * HID, e] = W2[e]
    b1c = np.ascontiguousarray(
        np.asarray(b1, np.float32).reshape(EH).reshape(MC, 128).T)
    b2rep = np.tile(np.asarray(b2, np.float32), 4).reshape(128, 1)
    selm = np.zeros((128, 4), np.float32)
    for j4 in range(4):
        selm[32 * j4:32 * (j4 + 1), j4] = 1.0
    return {"w1t": W1f.astype(bf), "wgt": wgt.astype(bf),
            "w2bd": w2bd.astype(bf), "b1c": b1c.astype(np.float32),
            "b2r": b2rep, "selm": selm.astype(bf)}


def layout_x(xc, np_dt, dma_group=8):
    """Core shard [B_LOC, D_IN] -> per-group contiguous transposed layout."""
    n_macro = xc.shape[0] // R
    G = min(dma_group, n_macro)
    n_group = n_macro // G
    return np.ascontiguousarray(
        xc.reshape(n_group, G, R, KC, 128).transpose(0, 4, 1, 3, 2).reshape(
            n_group * 128, G * KC * R)).astype(np_dt)


class Runner:
    """Reusable SPMD executor (multi-core path of run_bass_via_pjrt)."""

    def __init__(self, nc, n_cores=N_CORES):
        b2j.install_neuronx_cc_hook()
        self.nc = nc
        self.n_cores = n_cores
        partition_name = (
            nc.partition_id_tensor.name if nc.partition_id_tensor else None
        )
        in_names, out_names, out_avals, zero_outs = [], [], [], []
        for alloc in nc.m.functions[0].allocations:
            if not isinstance(alloc, mybir.MemoryLocationSet):
                continue
            assert alloc.memorylocations
            name = alloc.memorylocations[0].name
            if alloc.kind == "ExternalInput":
                if name != partition_name:
                    in_names.append(name)
            elif alloc.kind == "ExternalOutput":
                out_names.append(name)
                shape = tuple(alloc.tensor_shape)
                dtype = mybir.dt.np(alloc.dtype)
                out_avals.append(jax.core.ShapedArray(shape, dtype))
                zero_outs.append(np.zeros(shape, dtype))
        self.in_names = list(in_names)
        self.out_names = out_names
        self.zero_outs = zero_outs
        n_params = len(in_names)
        n_outs = len(out_names)
        bind_names = in_names + out_names
        if partition_name is not None:
            bind_names.append(partition_name)

        def _body(*args):
            operands = list(args)
            if partition_name is not None:
                operands.append(b2j.partition_id_tensor())
            outs = b2j._bass_exec_p.bind(
                *operands,
                out_avals=tuple(out_avals),
                in_names=tuple(bind_names),
                out_names=tuple(out_names),
                lowering_input_output_aliases=(),
                sim_require_finite=True,
                sim_require_nnan=True,
                nc=nc,
            )
            return tuple(outs)

        devices = jax.devices()[:n_cores]
        assert len(devices) == n_cores
        self.mesh = Mesh(np.asarray(devices), ("core",))
        in_specs = (PartitionSpec("core"),) * (n_params + n_outs)
        out_specs = (PartitionSpec("core"),) * n_outs
        self.fn = jax.jit(
            shard_map(_body, mesh=self.mesh, in_specs=in_specs,
                      out_specs=out_specs, check_rep=False),
            donate_argnums=tuple(range(n_params, n_params + n_outs)),
            keep_unused=True,
        )
        self.sharding = NamedSharding(self.mesh, PartitionSpec("core"))

    def put_inputs(self, in_maps):
        assert len(in_maps) == self.n_cores
        concat = [
            np.concatenate([np.asarray(m[name]) for m in in_maps], axis=0)
            for name in self.in_names
        ]
        return [jax.device_put(a, self.sharding) for a in concat]

    def fresh_outs(self):
        return [
            jax.device_put(
                np.zeros((self.n_cores * z.shape[0], *z.shape[1:]), z.dtype),
                self.sharding,
            )
            for z in self.zero_outs
        ]

    def run(self, dev_inputs, dev_outs=None):
        if dev_outs is None:
            dev_outs = self.fresh_outs()
        return self.fn(*dev_inputs, *dev_outs)


def get_runner(b_loc=B_LOC):
    if b_loc not in _RUNNER_CACHE:
        if b_loc not in _NC_CACHE:
            _NC_CACHE[b_loc] = build_nc(b_loc)
        _RUNNER_CACHE[b_loc] = Runner(_NC_CACHE[b_loc])
    return _RUNNER_CACHE[b_loc]


def make_in_maps(x, Wg, W1, b1, W2, b2, dma_group=8):
    import ml_dtypes
    x = np.asarray(x, dtype=np.float32)
    consts = prep_weights(Wg, W1, b1, W2, b2)
    xs = x.reshape(N_CORES, B_LOC, D_IN)
    in_maps = []
    for i in range(N_CORES):
        m = dict(consts)
        m["xTm"] = layout_x(xs[i], ml_dtypes.bfloat16, dma_group)
        in_maps.append(m)
    return in_maps


def decode_out(out_t):
    """[N_CORES*4, n4*1024] f32 -> [BATCH, 1] f32 via host division."""
    n4 = B_LOC // R // 4
    o = out_t.reshape(N_CORES, 4, n4, 2, 512)
    den = o[:, :, :, 0, :]
    num = o[:, :, :, 1, :]
    xi = num / den  # [cores, j4, g4, col]
    xi = xi.transpose(0, 2, 1, 3).reshape(BATCH)
    return np.ascontiguousarray(xi[:, None]).astype(np.float32)


def kernel(x, Wg, W1, b1, W2, b2):
    os.environ["BASS_NEVER_TRACE"] = "1"
    in_maps = make_in_maps(x, Wg, W1, b1, W2, b2)
    runner = get_runner(B_LOC)
    dev_in = runner.put_inputs(in_maps)
    outs = runner.run(dev_in)
    return decode_out(np.asarray(outs[0]))


if __name__ == "__main__":
    rng = np.random.default_rng(0)
    inputs = {
        "x": rng.standard_normal((BATCH, D_IN), dtype=np.float32),
        "Wg": (rng.standard_normal((N_EXPERTS, D_IN)) * 0.02).astype(np.float32),
        "W1": (rng.standard_normal((N_EXPERTS, D_IN, HID)) * 0.02).astype(np.float32),
        "b1": (rng.standard_normal((N_EXPERTS, HID)) * 0.02).astype(np.float32),
        "W2": (rng.standard_normal((N_EXPERTS, HID)) * 0.02).astype(np.float32),
        "b2": (rng.standard_normal((N_EXPERTS,)) * 0.02).astype(np.float32),
    }
    out = kernel(**inputs)
    print(out.shape, out.dtype, out[:4, 0])


# revision 17
# speedup vs baseline: 1.2375x; 1.1593x over previous
"""Trainium2 Bass kernel for MoEPred: softmax-gated mixture of 32 tiny experts.

  xi[b] = sum_e softmax_e(x@Wg.T) * (W2[e] . gelu(x @ W1[e] + b1[e]) + b2[e])

Sharding: pure data parallel over batch across 8 NeuronCores; weights
replicated. x pre-laid-out on host so each 512-row macro-tile streams as xT
chunks [feat 128, rows 512] (contraction dim on SBUF partitions).

Key structure (per 512-row macro-tile, 64 per core):
  MM1   hT[eh,R] = W1.T @ xT      [PE]  mm1="bf16": 16 bf16 matmuls
                                        mm1="dr24": 24 fp8 DoubleRow passes
        dr24: W1*64 split into fp8 hi+lo (scale dodges e4m3 subnormals),
        x split into fp8 hi + (lo*16 vs W1hi/16); three k-paired DR sets give
        bf16-level accuracy at 0.6x the PE time per pass.
  gelu  ha = gelu(hT/S + b1)      4 ACT instrs (bias+descale fused)  [ACT]
  gate  g -> one shared PSUM bank for 4 macros via col groups        [PE]
  MM2   out2 -> shared bank, 4 macro col groups (bf16)               [PE]
Per 4-macro group (small ops run at full 128-partition width):
  softmax exp WITHOUT the exp table (exp and gelu never share an ACT
  table; each swap costs 1283ns):  t = tanh(g/(2S)) [ACT, gelu's table]
  exp(g) = (1+t)/(1-t)            [DVE]
  st2 = (out2 + b2) * exp         [DVE]
  num/den via two ones-matmuls -> [4,1024] psum -> SBUF -> DRAM; final
  xi = num/den division on HOST (avoids [1,R] 1-partition DVE ops).
"""

import os
import sys
from contextlib import ExitStack

import numpy as np

for _p in ("/opt/trn_rl_repo",):
    if _p not in sys.path:
        sys.path.insert(0, _p)

import jax
from jax.experimental.shard_map import shard_map
from jax.sharding import Mesh, NamedSharding, PartitionSpec

import concourse.bacc as bacc
import concourse.bass2jax as b2j
import concourse.tile as tile
from concourse import mybir

N_CORES = 8
BATCH = 262144
D_IN = 512
N_EXPERTS = 32
HID = 16
EH = N_EXPERTS * HID  # 512
B_LOC = BATCH // N_CORES  # 32768
R = 512  # rows per macro-tile
KC = D_IN // 128  # 4 feature chunks
MC = EH // 128  # 4 eh chunks
WS = 64.0  # fp8 weight pre-scale (keeps W1*WS out of e4m3 subnormals)
XS = 16.0  # fp8 x-residual pre-scale

F32 = mybir.dt.float32
BF16 = mybir.dt.bfloat16
FP8 = mybir.dt.float8e4
AF = mybir.ActivationFunctionType
ALU = mybir.AluOpType
DR = mybir.MatmulPerfMode.DoubleRow

MM1_MODE = os.environ.get("KMM1", "bf16")

_NC_CACHE = {}
_RUNNER_CACHE = {}


def build_nc(b_loc=B_LOC, loop_n=1, level=5, dma_group=4, xq_bufs=6,
             dma_rings=2, mm1=None, ha_bufs=2, v_bufs=2, psh_bufs=2,
             psg_bufs=2, pso_bufs=2):
    """level: 0=dma, 1=+mm1, 2=+gelu, 3=+gating/exp, 4=+mm2/stt, 5=full."""
    if mm1 is None:
        mm1 = MM1_MODE
    dr = mm1 == "dr24"
    assert b_loc % (4 * R) == 0
    n_macro = b_loc // R
    n4 = n_macro // 4
    G = min(dma_group, n_macro)
    assert n_macro % G == 0 and (G % 4 == 0 or 4 % G == 0)
    n_group = n_macro // G

    nc = bacc.Bacc("TRN2", target_bir_lowering=False, debug=False,
                   num_devices=N_CORES)

    if dr:
        xhi_d = nc.dram_tensor("xhi", [n_group * 128, G * KC * R], FP8,
                               kind="ExternalInput")
        xlo_d = nc.dram_tensor("xlo", [n_group * 128, G * KC * R], FP8,
                               kind="ExternalInput")
        w1s_d = nc.dram_tensor("w1s", [3 * D_IN, EH], FP8, kind="ExternalInput")
        wgs_d = nc.dram_tensor("wgs", [3 * D_IN, N_EXPERTS], FP8,
                               kind="ExternalInput")
    else:
        xTm = nc.dram_tensor("xTm", [n_group * 128, G * KC * R], BF16,
                             kind="ExternalInput")
        w1t = nc.dram_tensor("w1t", [D_IN, EH], BF16, kind="ExternalInput")
        wgt = nc.dram_tensor("wgt", [D_IN, N_EXPERTS], BF16,
                             kind="ExternalInput")
    w2bd = nc.dram_tensor("w2bd", [EH, N_EXPERTS], BF16, kind="ExternalInput")
    b1c = nc.dram_tensor("b1c", [128, MC], F32, kind="ExternalInput")
    b2r = nc.dram_tensor("b2r", [128, 1], F32, kind="ExternalInput")
    selm = nc.dram_tensor("selm", [128, 4], BF16, kind="ExternalInput")
    outT = nc.dram_tensor("outT", [4, n4 * 1024], F32, kind="ExternalOutput")

    with tile.TileContext(nc) as tc, ExitStack() as ctx:
        const = ctx.enter_context(tc.tile_pool(name="const", bufs=1))
        xpool = ctx.enter_context(tc.tile_pool(name="xp", bufs=xq_bufs))
        hapool = ctx.enter_context(tc.tile_pool(name="hap", bufs=ha_bufs))
        vpool = ctx.enter_context(tc.tile_pool(name="vp", bufs=v_bufs))
        cpool = ctx.enter_context(tc.tile_pool(name="cp", bufs=2))
        ps_h = ctx.enter_context(tc.tile_pool(name="ps_h", bufs=psh_bufs, space="PSUM"))
        ps_g = ctx.enter_context(tc.tile_pool(name="ps_g", bufs=psg_bufs, space="PSUM"))
        ps_o = ctx.enter_context(tc.tile_pool(name="ps_o", bufs=pso_bufs, space="PSUM"))
        ps_s = ctx.enter_context(tc.tile_pool(name="ps_s", bufs=1, space="PSUM"))

        # --- replicated constants, loaded once ---
        in_dt = FP8 if dr else BF16
        NW = 3 if dr else 1  # weight sets: hi, lo, hi/XS
        w1_sb = const.tile([128, NW, KC, EH], in_dt, name="w1_sb")
        wg_sb = const.tile([128, NW, KC, N_EXPERTS], in_dt, name="wg_sb")
        w2_sb = const.tile([128, MC, N_EXPERTS], BF16, name="w2_sb")
        b1_sb = const.tile([128, MC], F32, name="b1_sb")
        b2_sb = const.tile([128, 1], F32, name="b2_sb")
        sel_sb = const.tile([128, 4], BF16, name="sel_sb")
        for s in range(NW):
            for k in range(KC):
                r0 = s * D_IN + k * 128
                if dr:
                    nc.sync.dma_start(w1_sb[:, s, k, :], w1s_d[r0:r0 + 128, :])
                    nc.sync.dma_start(wg_sb[:, s, k, :], wgs_d[r0:r0 + 128, :])
                else:
                    nc.sync.dma_start(w1_sb[:, s, k, :], w1t[r0:r0 + 128, :])
                    nc.sync.dma_start(wg_sb[:, s, k, :], wgt[r0:r0 + 128, :])
        for k in range(KC):
            nc.sync.dma_start(w2_sb[:, k, :], w2bd[k * 128:(k + 1) * 128, :])
        nc.sync.dma_start(b1_sb[:], b1c[:, :])
        nc.sync.dma_start(b2_sb[:], b2r[:, :])
        nc.sync.dma_start(sel_sb[:], selm[:, :])

        if loop_n > 1:
            ctx.enter_context(tc.For_i(0, loop_n, 1))

        # group-DMA prefetch
        PREFETCH = max(0, min(xq_bufs - 2, 2))
        xq_tiles = {}

        def issue_group_dma(grp):
            if grp >= n_group:
                return
            rs = slice(grp * 128, (grp + 1) * 128)
            if dr:
                xh_t = xpool.tile([128, G, KC, R], FP8, tag="xh", name="xh")
                xl_t = xpool.tile([128, G, KC, R], FP8, tag="xl", name="xl")
                eng = nc.sync if (dma_rings == 1 or grp % 2 == 0) else nc.scalar
                eng2 = nc.scalar if (dma_rings == 1 or grp % 2 == 0) else nc.sync
                eng.dma_start(xh_t[:], xhi_d[rs, :])
                eng2.dma_start(xl_t[:], xlo_d[rs, :])
                xq_tiles[grp] = (xh_t, xl_t)
            else:
                xq_t = xpool.tile([128, G, KC, R], BF16, tag="xq", name="xq")
                eng = nc.sync if (dma_rings == 1 or grp % 2 == 0) else nc.scalar
                eng.dma_start(xq_t[:], xTm[rs, :])
                xq_tiles[grp] = (xq_t,)

        for g0 in range(PREFETCH + 1):
            issue_group_dma(g0)

        def emit_dr24(out_ap, wsb, xh, xl, nsl, **kw):
            """3 k-paired DR sets: (whi,xhi) (wlo,xhi) (whi/XS, xlo*XS)."""
            for s in range(3):
                xs = xh if s < 2 else xl
                ws = s if s < 2 else 2
                for kj in range(KC // 2):
                    nc.tensor.matmul(
                        out_ap, lhsT=wsb[:, ws, 2 * kj:2 * kj + 2, nsl],
                        rhs=xs[:, 2 * kj:2 * kj + 2, :],
                        start=(s == 0 and kj == 0),
                        stop=(s == 2 and kj == KC // 2 - 1),
                        perf_mode=DR, **kw)

        xconst = None
        if level == 11:  # const-x MM1-only diagnostic (no x-DMA dependency)
            xconst = const.tile([128, KC, R], in_dt, name="xconst")
            nc.vector.memset(xconst[:], 0.25)

        level_eff = 1 if level == 11 else level
        gb = ob = None
        pend_mm2 = []   # (j4, ob_tile, ha_tile): MM2 deferred one macro
        pend_tail = []  # (g4, gb, ob): group tail deferred two macros

        def emit_mm2():
            pj4, pob, pha = pend_mm2.pop(0)
            osl = pob[32 * pj4:32 * (pj4 + 1), :]
            for m in range(MC):
                nc.tensor.matmul(
                    osl, lhsT=w2_sb[:, m, :], rhs=pha[:, m, :],
                    start=(m == 0), stop=(m == MC - 1),
                    tile_position=(0, 32 * pj4),
                    skip_group_check=True)

        def emit_tail():
            pg4, pgb, pob = pend_tail.pop(0)
            if level_eff >= 3:
                # t = tanh(g/2); exp(g) = (1+t)/(1-t)  [no table swap]
                th = vpool.tile([128, R], F32, tag="th", name="th")
                nc.scalar.activation(th[:], pgb[:], AF.Tanh, bias=0.0,
                                     scale=(0.5 / WS) if dr else 0.5)
                dd = vpool.tile([128, R], F32, tag="dd", name="dd")
                nc.vector.tensor_scalar(dd[:], th[:], -1.0, 1.0,
                                        ALU.mult, ALU.add)
                rr = vpool.tile([128, R], F32, tag="rr", name="rr")
                nc.vector.reciprocal(rr[:], dd[:])
                exp4 = vpool.tile([128, R], BF16, tag="e4", name="e4")
                nc.vector.scalar_tensor_tensor(
                    exp4[:], th[:], 1.0, rr[:], ALU.add, ALU.mult)
            if level_eff >= 4:
                st2 = vpool.tile([128, R], BF16, tag="st2", name="st2")
                nc.vector.scalar_tensor_tensor(
                    st2[:], pob[:], b2_sb[:], exp4[:], ALU.add, ALU.mult)
            cp = cpool.tile([4, 1024], F32, tag="cpo", name="cpo")
            if level_eff >= 5:
                sp = ps_s.tile([4, 1024], F32, tag="sp", name="sp")
                nc.tensor.matmul(sp[:, 0:512], lhsT=sel_sb[:],
                                 rhs=exp4[:], start=True, stop=True,
                                 skip_group_check=True)
                nc.tensor.matmul(sp[:, 512:1024], lhsT=sel_sb[:],
                                 rhs=st2[:], start=True, stop=True,
                                 skip_group_check=True)
                nc.vector.tensor_scalar_add(cp[:], sp[:], 0.0)
            else:
                nc.vector.memset(cp[:], 1.0)
            nc.gpsimd.dma_start(
                outT[:, pg4 * 1024:(pg4 + 1) * 1024], cp[:])

        for j in range(n_macro):
            j4 = j % 4
            g4 = j // 4
            if j % G == 0:
                grp = j // G
                issue_group_dma(grp + PREFETCH + 1)
                xq = xq_tiles.pop(grp, None)
            if level == 11:
                xj = xjh = xjl = xconst
            elif dr:
                xjh = xq[0][:, j % G]
                xjl = xq[1][:, j % G]
            else:
                xj = xq[0][:, j % G]

            if level_eff >= 1:
                # MM1 + gelu per m-chunk (bias + 1/WS descale fused into ACT)
                ha = hapool.tile([128, MC, R], BF16, tag="ha", name="ha")
                for m in range(MC):
                    hp = ps_h.tile([128, R], F32, tag="hp", name="hp")
                    msl = slice(m * 128, (m + 1) * 128)
                    if dr:
                        emit_dr24(hp[:], w1_sb, xjh, xjl, msl)
                    else:
                        for k in range(KC):
                            nc.tensor.matmul(
                                hp[:], lhsT=w1_sb[:, 0, k, msl],
                                rhs=xj[:, k, :],
                                start=(k == 0), stop=(k == KC - 1))
                    if level_eff >= 2:
                        nc.scalar.activation(
                            ha[:, m, :], hp[:], AF.Gelu,
                            bias=b1_sb[:, m:m + 1],
                            scale=(1.0 / WS) if dr else 1.0)

            if level_eff >= 3:
                # gating for 4 macros shares one PSUM bank via col groups
                if j4 == 0:
                    gb = ps_g.tile([128, R], F32, tag="gb", name="gb")
                gsl = gb[32 * j4:32 * (j4 + 1), :]
                if dr:
                    emit_dr24(gsl, wg_sb, xjh, xjl, slice(0, N_EXPERTS),
                              tile_position=(0, 32 * j4),
                              skip_group_check=True)
                else:
                    for k in range(KC):
                        nc.tensor.matmul(
                            gsl, lhsT=wg_sb[:, 0, k, :], rhs=xj[:, k, :],
                            start=(k == 0), stop=(k == KC - 1),
                            tile_position=(0, 32 * j4),
                            skip_group_check=True)

            if level_eff >= 4:
                if j4 == 0:
                    ob = ps_o.tile([128, R], F32, tag="ob", name="ob")
                if pend_mm2:
                    emit_mm2()
                if pend_tail:
                    emit_tail()
                pend_mm2.append((j4, ob, ha))
                if j4 == 3:
                    pend_tail.append((g4, gb, ob))
            elif j4 == 3:
                pend_tail.append((g4, gb, ob))
                if level_eff < 4:
                    emit_tail()

        while pend_mm2:
            emit_mm2()
        while pend_tail:
            emit_tail()

    nc.compile()
    return nc


def _split8(a, scale, rescale):
    """a*scale -> (hi8, lo8, hi8/rescale) stacked along rows, f32 staging."""
    import ml_dtypes
    f8 = ml_dtypes.float8_e4m3
    asc = np.asarray(a, np.float32) * scale
    hi = asc.astype(f8)
    lo = (asc - hi.astype(np.float32)).astype(f8)
    h16 = (asc / rescale).astype(f8)
    return np.concatenate(
        [hi.astype(np.float32), lo.astype(np.float32), h16.astype(np.float32)],
        axis=0)


def prep_weights(Wg, W1, b1, W2, b2, mm1=None):
    import ml_dtypes
    if mm1 is None:
        mm1 = MM1_MODE
    bf = ml_dtypes.bfloat16
    f8 = ml_dtypes.float8_e4m3
    W1f = np.ascontiguousarray(
        np.asarray(W1, np.float32).transpose(1, 0, 2).reshape(D_IN, EH))
    wgt = np.ascontiguousarray(np.asarray(Wg, np.float32).T)
    w2bd = np.zeros((EH, N_EXPERTS), np.float32)
    W2 = np.asarray(W2, np.float32)
    for e in range(N_EXPERTS):
        w2bd[e * HID:(e + 1) * HID, e] = W2[e]
    b1c = np.ascontiguousarray(
        np.asarray(b1, np.float32).reshape(EH).reshape(MC, 128).T)
    b2rep = np.tile(np.asarray(b2, np.float32), 4).reshape(128, 1)
    selm = np.zeros((128, 4), np.float32)
    for j4 in range(4):
        selm[32 * j4:32 * (j4 + 1), j4] = 1.0
    out = {"w2bd": w2bd.astype(bf), "b1c": b1c.astype(np.float32),
           "b2r": b2rep, "selm": selm.astype(bf)}
    if mm1 == "dr24":
        out["w1s"] = _split8(W1f, WS, XS).astype(f8)
        out["wgs"] = _split8(wgt, WS, XS).astype(f8)
    else:
        out["w1t"] = W1f.astype(bf)
        out["wgt"] = wgt.astype(bf)
    return out


def layout_x(xc, np_dt, dma_group=8):
    """Core shard [B_LOC, D_IN] -> per-group contiguous transposed layout."""
    n_macro = xc.shape[0] // R
    G = min(dma_group, n_macro)
    n_group = n_macro // G
    return np.ascontiguousarray(
        xc.reshape(n_group, G, R, KC, 128).transpose(0, 4, 1, 3, 2).reshape(
            n_group * 128, G * KC * R)).astype(np_dt)


class Runner:
    """Reusable SPMD executor (multi-core path of run_bass_via_pjrt)."""

    def __init__(self, nc, n_cores=N_CORES):
        b2j.install_neuronx_cc_hook()
        self.nc = nc
        self.n_cores = n_cores
        partition_name = (
            nc.partition_id_tensor.name if nc.partition_id_tensor else None
        )
        in_names, out_names, out_avals, zero_outs = [], [], [], []
        for alloc in nc.m.functions[0].allocations:
            if not isinstance(alloc, mybir.MemoryLocationSet):
                continue
            assert alloc.memorylocations
            name = alloc.memorylocations[0].name
            if alloc.kind == "ExternalInput":
                if name != partition_name:
                    in_names.append(name)
            elif alloc.kind == "ExternalOutput":
                out_names.append(name)
                shape = tuple(alloc.tensor_shape)
                dtype = mybir.dt.np(alloc.dtype)
                out_avals.append(jax.core.ShapedArray(shape, dtype))
                zero_outs.append(np.zeros(shape, dtype))
        self.in_names = list(in_names)
        self.out_names = out_names
        self.zero_outs = zero_outs
        n_params = len(in_names)
        n_outs = len(out_names)
        bind_names = in_names + out_names
        if partition_name is not None:
            bind_names.append(partition_name)

        def _body(*args):
            operands = list(args)
            if partition_name is not None:
                operands.append(b2j.partition_id_tensor())
            outs = b2j._bass_exec_p.bind(
                *operands,
                out_avals=tuple(out_avals),
                in_names=tuple(bind_names),
                out_names=tuple(out_names),
                lowering_input_output_aliases=(),
                sim_require_finite=True,
                sim_require_nnan=True,
                nc=nc,
            )
            return tuple(outs)

        devices = jax.devices()[:n_cores]
        assert len(devices) == n_cores
        self.mesh = Mesh(np.asarray(devices), ("core",))
        in_specs = (PartitionSpec("core"),) * (n_params + n_outs)
        out_specs = (PartitionSpec("core"),) * n_outs
        self.fn = jax.jit(
            shard_map(_body, mesh=self.mesh, in_specs=in_specs,
                      out_specs=out_specs, check_rep=False),
            donate_argnums=tuple(range(n_params, n_params + n_outs)),
            keep_unused=True,
        )
        self.sharding = NamedSharding(self.mesh, PartitionSpec("core"))

    def put_inputs(self, in_maps):
        assert len(in_maps) == self.n_cores
        concat = [
            np.concatenate([np.asarray(m[name]) for m in in_maps], axis=0)
            for name in self.in_names
        ]
        return [jax.device_put(a, self.sharding) for a in concat]

    def fresh_outs(self):
        return [
            jax.device_put(
                np.zeros((self.n_cores * z.shape[0], *z.shape[1:]), z.dtype),
                self.sharding,
            )
            for z in self.zero_outs
        ]

    def run(self, dev_inputs, dev_outs=None):
        if dev_outs is None:
            dev_outs = self.fresh_outs()
        return self.fn(*dev_inputs, *dev_outs)


def get_runner(b_loc=B_LOC):
    if b_loc not in _RUNNER_CACHE:
        if b_loc not in _NC_CACHE:
            _NC_CACHE[b_loc] = build_nc(b_loc)
        _RUNNER_CACHE[b_loc] = Runner(_NC_CACHE[b_loc])
    return _RUNNER_CACHE[b_loc]


def make_in_maps(x, Wg, W1, b1, W2, b2, dma_group=4, mm1=None):
    import ml_dtypes
    if mm1 is None:
        mm1 = MM1_MODE
    f8 = ml_dtypes.float8_e4m3
    x = np.asarray(x, dtype=np.float32)
    consts = prep_weights(Wg, W1, b1, W2, b2, mm1)
    xs = x.reshape(N_CORES, B_LOC, D_IN)
    in_maps = []
    for i in range(N_CORES):
        m = dict(consts)
        if mm1 == "dr24":
            xhi = xs[i].astype(f8)
            xlo = ((xs[i] - xhi.astype(np.float32)) * XS).astype(f8)
            m["xhi"] = layout_x(xhi.astype(np.float32), f8, dma_group)
            m["xlo"] = layout_x(xlo.astype(np.float32), f8, dma_group)
        else:
            m["xTm"] = layout_x(xs[i], ml_dtypes.bfloat16, dma_group)
        in_maps.append(m)
    return in_maps


def decode_out(out_t):
    """[N_CORES*4, n4*1024] f32 -> [BATCH, 1] f32 via host division."""
    n4 = B_LOC // R // 4
    o = out_t.reshape(N_CORES, 4, n4, 2, 512)
    den = o[:, :, :, 0, :]
    num = o[:, :, :, 1, :]
    xi = num / den  # [cores, j4, g4, col]
    xi = xi.transpose(0, 2, 1, 3).reshape(BATCH)
    return np.ascontiguousarray(xi[:, None]).astype(np.float32)


def kernel(x, Wg, W1, b1, W2, b2):
    os.environ["BASS_NEVER_TRACE"] = "1"
    in_maps = make_in_maps(x, Wg, W1, b1, W2, b2)
    runner = get_runner(B_LOC)
    dev_in = runner.put_inputs(in_maps)
    outs = runner.run(dev_in)
    return decode_out(np.asarray(outs[0]))


if __name__ == "__main__":
    rng = np.random.default_rng(0)
    inputs = {
        "x": rng.standard_normal((BATCH, D_IN), dtype=np.float32),
        "Wg": (rng.standard_normal((N_EXPERTS, D_IN)) * 0.02).astype(np.float32),
        "W1": (rng.standard_normal((N_EXPERTS, D_IN, HID)) * 0.02).astype(np.float32),
        "b1": (rng.standard_normal((N_EXPERTS, HID)) * 0.02).astype(np.float32),
        "W2": (rng.standard_normal((N_EXPERTS, HID)) * 0.02).astype(np.float32),
        "b2": (rng.standard_normal((N_EXPERTS,)) * 0.02).astype(np.float32),
    }
    out = kernel(**inputs)
    print(out.shape, out.dtype, out[:4, 0])


# revision 18
# speedup vs baseline: 1.2653x; 1.0225x over previous
"""Trainium2 Bass kernel for MoEPred: softmax-gated mixture of 32 tiny experts.

  xi[b] = sum_e softmax_e(x@Wg.T) * (W2[e] . gelu(x @ W1[e] + b1[e]) + b2[e])

Sharding: pure data parallel over batch across 8 NeuronCores; weights
replicated. x pre-laid-out on host so each 512-row macro-tile streams as xT
chunks [feat 128, rows 512] (contraction dim on SBUF partitions).

Key structure (per 512-row macro-tile, 64 per core):
  MM1   hT[eh,R] = W1.T @ xT      [PE]  mm1="bf16": 16 bf16 matmuls
                                        mm1="dr24": 24 fp8 DoubleRow passes
        dr24: W1*64 split into fp8 hi+lo (scale dodges e4m3 subnormals),
        x split into fp8 hi + (lo*16 vs W1hi/16); three k-paired DR sets give
        bf16-level accuracy at 0.6x the PE time per pass.
  gelu  ha = gelu(hT/S + b1)      4 ACT instrs (bias+descale fused)  [ACT]
  gate  g -> one shared PSUM bank for 4 macros via col groups        [PE]
  MM2   out2 -> shared bank, 4 macro col groups (bf16)               [PE]
Per 4-macro group (small ops run at full 128-partition width):
  softmax exp WITHOUT the exp table (exp and gelu never share an ACT
  table; each swap costs 1283ns):  t = tanh(g/(2S)) [ACT, gelu's table]
  exp(g) = (1+t)/(1-t)            [DVE]
  st2 = (out2 + b2) * exp         [DVE]
  num/den via two ones-matmuls -> [4,1024] psum -> SBUF -> DRAM; final
  xi = num/den division on HOST (avoids [1,R] 1-partition DVE ops).
"""

import os
import sys
from contextlib import ExitStack

import numpy as np

for _p in ("/opt/trn_rl_repo",):
    if _p not in sys.path:
        sys.path.insert(0, _p)

import jax
from jax.experimental.shard_map import shard_map
from jax.sharding import Mesh, NamedSharding, PartitionSpec

import concourse.bacc as bacc
import concourse.bass2jax as b2j
import concourse.tile as tile
from concourse import mybir

N_CORES = 8
BATCH = 262144
D_IN = 512
N_EXPERTS = 32
HID = 16
EH = N_EXPERTS * HID  # 512
B_LOC = BATCH // N_CORES  # 32768
R = 512  # rows per macro-tile
KC = D_IN // 128  # 4 feature chunks
MC = EH // 128  # 4 eh chunks
WS = 64.0  # fp8 weight pre-scale (keeps W1*WS out of e4m3 subnormals)
XS = 16.0  # fp8 x-residual pre-scale

F32 = mybir.dt.float32
BF16 = mybir.dt.bfloat16
FP8 = mybir.dt.float8e4
AF = mybir.ActivationFunctionType
ALU = mybir.AluOpType
DR = mybir.MatmulPerfMode.DoubleRow

MM1_MODE = os.environ.get("KMM1", "bf16")

_NC_CACHE = {}
_RUNNER_CACHE = {}


def build_nc(b_loc=B_LOC, loop_n=1, level=5, dma_group=4, xq_bufs=6,
             dma_rings=2, mm1=None, ha_bufs=2, v_bufs=2, psh_bufs=2,
             psg_bufs=2, pso_bufs=2, tail_at=0, prefetch=None):
    """level: 0=dma, 1=+mm1, 2=+gelu, 3=+gating/exp, 4=+mm2/stt, 5=full."""
    if mm1 is None:
        mm1 = MM1_MODE
    dr = mm1 == "dr24"
    assert b_loc % (4 * R) == 0
    n_macro = b_loc // R
    n4 = n_macro // 4
    G = min(dma_group, n_macro)
    assert n_macro % G == 0 and (G % 4 == 0 or 4 % G == 0)
    n_group = n_macro // G

    nc = bacc.Bacc("TRN2", target_bir_lowering=False, debug=False,
                   num_devices=N_CORES)

    if dr:
        xhi_d = nc.dram_tensor("xhi", [n_group * 128, G * KC * R], FP8,
                               kind="ExternalInput")
        xlo_d = nc.dram_tensor("xlo", [n_group * 128, G * KC * R], FP8,
                               kind="ExternalInput")
        w1s_d = nc.dram_tensor("w1s", [3 * D_IN, EH], FP8, kind="ExternalInput")
        wgs_d = nc.dram_tensor("wgs", [3 * D_IN, N_EXPERTS], FP8,
                               kind="ExternalInput")
    else:
        xTm = nc.dram_tensor("xTm", [n_group * 128, G * KC * R], BF16,
                             kind="ExternalInput")
        w1t = nc.dram_tensor("w1t", [D_IN, EH], BF16, kind="ExternalInput")
        wgt = nc.dram_tensor("wgt", [D_IN, N_EXPERTS], BF16,
                             kind="ExternalInput")
    w2bd = nc.dram_tensor("w2bd", [EH, N_EXPERTS], BF16, kind="ExternalInput")
    b1c = nc.dram_tensor("b1c", [128, MC], F32, kind="ExternalInput")
    b2r = nc.dram_tensor("b2r", [128, 1], F32, kind="ExternalInput")
    selm = nc.dram_tensor("selm", [128, 4], BF16, kind="ExternalInput")
    outT = nc.dram_tensor("outT", [4, n4 * 1024], F32, kind="ExternalOutput")

    with tile.TileContext(nc) as tc, ExitStack() as ctx:
        const = ctx.enter_context(tc.tile_pool(name="const", bufs=1))
        xpool = ctx.enter_context(tc.tile_pool(name="xp", bufs=xq_bufs))
        hapool = ctx.enter_context(tc.tile_pool(name="hap", bufs=ha_bufs))
        vpool = ctx.enter_context(tc.tile_pool(name="vp", bufs=v_bufs))
        cpool = ctx.enter_context(tc.tile_pool(name="cp", bufs=2))
        ps_h = ctx.enter_context(tc.tile_pool(name="ps_h", bufs=psh_bufs, space="PSUM"))
        ps_g = ctx.enter_context(tc.tile_pool(name="ps_g", bufs=psg_bufs, space="PSUM"))
        ps_o = ctx.enter_context(tc.tile_pool(name="ps_o", bufs=pso_bufs, space="PSUM"))
        ps_s = ctx.enter_context(tc.tile_pool(name="ps_s", bufs=1, space="PSUM"))

        # --- replicated constants, loaded once ---
        in_dt = FP8 if dr else BF16
        NW = 3 if dr else 1  # weight sets: hi, lo, hi/XS
        w1_sb = const.tile([128, NW, KC, EH], in_dt, name="w1_sb")
        wg_sb = const.tile([128, NW, KC, N_EXPERTS], in_dt, name="wg_sb")
        w2_sb = const.tile([128, MC, N_EXPERTS], BF16, name="w2_sb")
        b1_sb = const.tile([128, MC], F32, name="b1_sb")
        b2_sb = const.tile([128, 1], F32, name="b2_sb")
        sel_sb = const.tile([128, 4], BF16, name="sel_sb")
        for s in range(NW):
            for k in range(KC):
                r0 = s * D_IN + k * 128
                if dr:
                    nc.sync.dma_start(w1_sb[:, s, k, :], w1s_d[r0:r0 + 128, :])
                    nc.sync.dma_start(wg_sb[:, s, k, :], wgs_d[r0:r0 + 128, :])
                else:
                    nc.sync.dma_start(w1_sb[:, s, k, :], w1t[r0:r0 + 128, :])
                    nc.sync.dma_start(wg_sb[:, s, k, :], wgt[r0:r0 + 128, :])
        for k in range(KC):
            nc.sync.dma_start(w2_sb[:, k, :], w2bd[k * 128:(k + 1) * 128, :])
        nc.sync.dma_start(b1_sb[:], b1c[:, :])
        nc.sync.dma_start(b2_sb[:], b2r[:, :])
        nc.sync.dma_start(sel_sb[:], selm[:, :])

        if loop_n > 1:
            ctx.enter_context(tc.For_i(0, loop_n, 1))

        # group-DMA prefetch
        PREFETCH = max(0, min(xq_bufs - 2, 2) if prefetch is None else prefetch)
        xq_tiles = {}

        def issue_group_dma(grp):
            if grp >= n_group:
                return
            rs = slice(grp * 128, (grp + 1) * 128)
            if dr:
                xh_t = xpool.tile([128, G, KC, R], FP8, tag="xh", name="xh")
                xl_t = xpool.tile([128, G, KC, R], FP8, tag="xl", name="xl")
                eng = nc.sync if (dma_rings == 1 or grp % 2 == 0) else nc.scalar
                eng2 = nc.scalar if (dma_rings == 1 or grp % 2 == 0) else nc.sync
                eng.dma_start(xh_t[:], xhi_d[rs, :])
                eng2.dma_start(xl_t[:], xlo_d[rs, :])
                xq_tiles[grp] = (xh_t, xl_t)
            else:
                xq_t = xpool.tile([128, G, KC, R], BF16, tag="xq", name="xq")
                eng = nc.sync if (dma_rings == 1 or grp % 2 == 0) else nc.scalar
                eng.dma_start(xq_t[:], xTm[rs, :])
                xq_tiles[grp] = (xq_t,)

        for g0 in range(PREFETCH + 1):
            issue_group_dma(g0)

        def emit_dr24(out_ap, wsb, xh, xl, nsl, **kw):
            """3 k-paired DR sets: (whi,xhi) (wlo,xhi) (whi/XS, xlo*XS)."""
            for s in range(3):
                xs = xh if s < 2 else xl
                ws = s if s < 2 else 2
                for kj in range(KC // 2):
                    nc.tensor.matmul(
                        out_ap, lhsT=wsb[:, ws, 2 * kj:2 * kj + 2, nsl],
                        rhs=xs[:, 2 * kj:2 * kj + 2, :],
                        start=(s == 0 and kj == 0),
                        stop=(s == 2 and kj == KC // 2 - 1),
                        perf_mode=DR, **kw)

        xconst = None
        if level == 11:  # const-x MM1-only diagnostic (no x-DMA dependency)
            xconst = const.tile([128, KC, R], in_dt, name="xconst")
            nc.vector.memset(xconst[:], 0.25)

        level_eff = 1 if level == 11 else level
        gb = ob = None
        pend_mm2 = []   # (j4, ob_tile, ha_tile): MM2 deferred one macro
        pend_tail = []  # (g4, gb, ob): group tail deferred two macros

        def emit_mm2():
            pj4, pob, pha = pend_mm2.pop(0)
            osl = pob[32 * pj4:32 * (pj4 + 1), :]
            for m in range(MC):
                nc.tensor.matmul(
                    osl, lhsT=w2_sb[:, m, :], rhs=pha[:, m, :],
                    start=(m == 0), stop=(m == MC - 1),
                    tile_position=(0, 32 * pj4),
                    skip_group_check=True)

        def emit_tail():
            pg4, pgb, pob = pend_tail.pop(0)
            if level_eff >= 3:
                # t = tanh(g/2); exp(g) = (1+t)/(1-t)  [no table swap]
                th = vpool.tile([128, R], F32, tag="th", name="th")
                nc.scalar.activation(th[:], pgb[:], AF.Tanh, bias=0.0,
                                     scale=(0.5 / WS) if dr else 0.5)
                dd = vpool.tile([128, R], F32, tag="dd", name="dd")
                nc.vector.tensor_scalar(dd[:], th[:], -1.0, 1.0,
                                        ALU.mult, ALU.add)
                rr = vpool.tile([128, R], F32, tag="rr", name="rr")
                nc.vector.reciprocal(rr[:], dd[:])
                exp4 = vpool.tile([128, R], BF16, tag="e4", name="e4")
                nc.vector.scalar_tensor_tensor(
                    exp4[:], th[:], 1.0, rr[:], ALU.add, ALU.mult)
            if level_eff >= 4:
                st2 = vpool.tile([128, R], BF16, tag="st2", name="st2")
                nc.vector.scalar_tensor_tensor(
                    st2[:], pob[:], b2_sb[:], exp4[:], ALU.add, ALU.mult)
            cp = cpool.tile([4, 1024], F32, tag="cpo", name="cpo")
            if level_eff >= 5:
                sp = ps_s.tile([4, 1024], F32, tag="sp", name="sp")
                nc.tensor.matmul(sp[:, 0:512], lhsT=sel_sb[:],
                                 rhs=exp4[:], start=True, stop=True,
                                 skip_group_check=True)
                nc.tensor.matmul(sp[:, 512:1024], lhsT=sel_sb[:],
                                 rhs=st2[:], start=True, stop=True,
                                 skip_group_check=True)
                nc.vector.tensor_scalar_add(cp[:], sp[:], 0.0)
            else:
                nc.vector.memset(cp[:], 1.0)
            nc.gpsimd.dma_start(
                outT[:, pg4 * 1024:(pg4 + 1) * 1024], cp[:])

        for j in range(n_macro):
            j4 = j % 4
            g4 = j // 4
            if j % G == 0:
                grp = j // G
                issue_group_dma(grp + PREFETCH + 1)
                xq = xq_tiles.pop(grp, None)
            if level == 11:
                xj = xjh = xjl = xconst
            elif dr:
                xjh = xq[0][:, j % G]
                xjl = xq[1][:, j % G]
            else:
                xj = xq[0][:, j % G]

            if level_eff >= 1:
                # MM1 + gelu per m-chunk (bias + 1/WS descale fused into ACT)
                ha = hapool.tile([128, MC, R], BF16, tag="ha", name="ha")
                for m in range(MC):
                    hp = ps_h.tile([128, R], F32, tag="hp", name="hp")
                    msl = slice(m * 128, (m + 1) * 128)
                    if dr:
                        emit_dr24(hp[:], w1_sb, xjh, xjl, msl)
                    else:
                        for k in range(KC):
                            nc.tensor.matmul(
                                hp[:], lhsT=w1_sb[:, 0, k, msl],
                                rhs=xj[:, k, :],
                                start=(k == 0), stop=(k == KC - 1))
                    if level_eff >= 2:
                        nc.scalar.activation(
                            ha[:, m, :], hp[:], AF.Gelu,
                            bias=b1_sb[:, m:m + 1],
                            scale=(1.0 / WS) if dr else 1.0)

            if level_eff >= 3:
                # gating for 4 macros shares one PSUM bank via col groups
                if j4 == 0:
                    gb = ps_g.tile([128, R], F32, tag="gb", name="gb")
                gsl = gb[32 * j4:32 * (j4 + 1), :]
                if dr:
                    emit_dr24(gsl, wg_sb, xjh, xjl, slice(0, N_EXPERTS),
                              tile_position=(0, 32 * j4),
                              skip_group_check=True)
                else:
                    for k in range(KC):
                        nc.tensor.matmul(
                            gsl, lhsT=wg_sb[:, 0, k, :], rhs=xj[:, k, :],
                            start=(k == 0), stop=(k == KC - 1),
                            tile_position=(0, 32 * j4),
                            skip_group_check=True)

            if level_eff >= 4:
                if j4 == 0:
                    ob = ps_o.tile([128, R], F32, tag="ob", name="ob")
                if pend_mm2:
                    emit_mm2()
                if pend_tail and tail_at == 0:
                    emit_tail()
                pend_mm2.append((j4, ob, ha))
                if j4 == 3:
                    pend_tail.append((g4, gb, ob))
                if pend_tail and j4 == tail_at and j >= 4 and tail_at > 0:
                    emit_tail()
            elif j4 == 3:
                pend_tail.append((g4, gb, ob))
                if level_eff < 4:
                    emit_tail()

        while pend_mm2:
            emit_mm2()
        while pend_tail:
            emit_tail()

    nc.compile()
    return nc


def _split8(a, scale, rescale):
    """a*scale -> (hi8, lo8, hi8/rescale) stacked along rows, f32 staging."""
    import ml_dtypes
    f8 = ml_dtypes.float8_e4m3
    asc = np.asarray(a, np.float32) * scale
    hi = asc.astype(f8)
    lo = (asc - hi.astype(np.float32)).astype(f8)
    h16 = (asc / rescale).astype(f8)
    return np.concatenate(
        [hi.astype(np.float32), lo.astype(np.float32), h16.astype(np.float32)],
        axis=0)


def prep_weights(Wg, W1, b1, W2, b2, mm1=None):
    import ml_dtypes
    if mm1 is None:
        mm1 = MM1_MODE
    bf = ml_dtypes.bfloat16
    f8 = ml_dtypes.float8_e4m3
    W1f = np.ascontiguousarray(
        np.asarray(W1, np.float32).transpose(1, 0, 2).reshape(D_IN, EH))
    wgt = np.ascontiguousarray(np.asarray(Wg, np.float32).T)
    w2bd = np.zeros((EH, N_EXPERTS), np.float32)
    W2 = np.asarray(W2, np.float32)
    for e in range(N_EXPERTS):
        w2bd[e * HID:(e + 1) * HID, e] = W2[e]
    b1c = np.ascontiguousarray(
        np.asarray(b1, np.float32).reshape(EH).reshape(MC, 128).T)
    b2rep = np.tile(np.asarray(b2, np.float32), 4).reshape(128, 1)
    selm = np.zeros((128, 4), np.float32)
    for j4 in range(4):
        selm[32 * j4:32 * (j4 + 1), j4] = 1.0
    out = {"w2bd": w2bd.astype(bf), "b1c": b1c.astype(np.float32),
           "b2r": b2rep, "selm": selm.astype(bf)}
    if mm1 == "dr24":
        out["w1s"] = _split8(W1f, WS, XS).astype(f8)
        out["wgs"] = _split8(wgt, WS, XS).astype(f8)
    else:
        out["w1t"] = W1f.astype(bf)
        out["wgt"] = wgt.astype(bf)
    return out


def layout_x(xc, np_dt, dma_group=8):
    """Core shard [B_LOC, D_IN] -> per-group contiguous transposed layout."""
    n_macro = xc.shape[0] // R
    G = min(dma_group, n_macro)
    n_group = n_macro // G
    return np.ascontiguousarray(
        xc.reshape(n_group, G, R, KC, 128).transpose(0, 4, 1, 3, 2).reshape(
            n_group * 128, G * KC * R)).astype(np_dt)


class Runner:
    """Reusable SPMD executor (multi-core path of run_bass_via_pjrt)."""

    def __init__(self, nc, n_cores=N_CORES):
        b2j.install_neuronx_cc_hook()
        self.nc = nc
        self.n_cores = n_cores
        partition_name = (
            nc.partition_id_tensor.name if nc.partition_id_tensor else None
        )
        in_names, out_names, out_avals, zero_outs = [], [], [], []
        for alloc in nc.m.functions[0].allocations:
            if not isinstance(alloc, mybir.MemoryLocationSet):
                continue
            assert alloc.memorylocations
            name = alloc.memorylocations[0].name
            if alloc.kind == "ExternalInput":
                if name != partition_name:
                    in_names.append(name)
            elif alloc.kind == "ExternalOutput":
                out_names.append(name)
                shape = tuple(alloc.tensor_shape)
                dtype = mybir.dt.np(alloc.dtype)
                out_avals.append(jax.core.ShapedArray(shape, dtype))
                zero_outs.append(np.zeros(shape, dtype))
        self.in_names = list(in_names)
        self.out_names = out_names
        self.zero_outs = zero_outs
        n_params = len(in_names)
        n_outs = len(out_names)
        bind_names = in_names + out_names
        if partition_name is not None:
            bind_names.append(partition_name)

        def _body(*args):
            operands = list(args)
            if partition_name is not None:
                operands.append(b2j.partition_id_tensor())
            outs = b2j._bass_exec_p.bind(
                *operands,
                out_avals=tuple(out_avals),
                in_names=tuple(bind_names),
                out_names=tuple(out_names),
                lowering_input_output_aliases=(),
                sim_require_finite=True,
                sim_require_nnan=True,
                nc=nc,
            )
            return tuple(outs)

        devices = jax.devices()[:n_cores]
        assert len(devices) == n_cores
        self.mesh = Mesh(np.asarray(devices), ("core",))
        in_specs = (PartitionSpec("core"),) * (n_params + n_outs)
        out_specs = (PartitionSpec("core"),) * n_outs
        self.fn = jax.jit(
            shard_map(_body, mesh=self.mesh, in_specs=in_specs,
                      out_specs=out_specs, check_rep=False),
            donate_argnums=tuple(range(n_params, n_params + n_outs)),
            keep_unused=True,
        )
        self.sharding = NamedSharding(self.mesh, PartitionSpec("core"))

    def put_inputs(self, in_maps):
        assert len(in_maps) == self.n_cores
        concat = [
            np.concatenate([np.asarray(m[name]) for m in in_maps], axis=0)
            for name in self.in_names
        ]
        return [jax.device_put(a, self.sharding) for a in concat]

    def fresh_outs(self):
        return [
            jax.device_put(
                np.zeros((self.n_cores * z.shape[0], *z.shape[1:]), z.dtype),
                self.sharding,
            )
            for z in self.zero_outs
        ]

    def run(self, dev_inputs, dev_outs=None):
        if dev_outs is None:
            dev_outs = self.fresh_outs()
        return self.fn(*dev_inputs, *dev_outs)


def get_runner(b_loc=B_LOC):
    if b_loc not in _RUNNER_CACHE:
        if b_loc not in _NC_CACHE:
            _NC_CACHE[b_loc] = build_nc(b_loc)
        _RUNNER_CACHE[b_loc] = Runner(_NC_CACHE[b_loc])
    return _RUNNER_CACHE[b_loc]


def make_in_maps(x, Wg, W1, b1, W2, b2, dma_group=4, mm1=None):
    import ml_dtypes
    if mm1 is None:
        mm1 = MM1_MODE
    f8 = ml_dtypes.float8_e4m3
    x = np.asarray(x, dtype=np.float32)
    consts = prep_weights(Wg, W1, b1, W2, b2, mm1)
    xs = x.reshape(N_CORES, B_LOC, D_IN)
    in_maps = []
    for i in range(N_CORES):
        m = dict(consts)
        if mm1 == "dr24":
            xhi = xs[i].astype(f8)
            xlo = ((xs[i] - xhi.astype(np.float32)) * XS).astype(f8)
            m["xhi"] = layout_x(xhi.astype(np.float32), f8, dma_group)
            m["xlo"] = layout_x(xlo.astype(np.float32), f8, dma_group)
        else:
            m["xTm"] = layout_x(xs[i], ml_dtypes.bfloat16, dma_group)
        in_maps.append(m)
    return in_maps


def decode_out(out_t):
    """[N_CORES*4, n4*1024] f32 -> [BATCH, 1] f32 via host division."""
    n4 = B_LOC // R // 4
    o = out_t.reshape(N_CORES, 4, n4, 2, 512)
    den = o[:, :, :, 0, :]
    num = o[:, :, :, 1, :]
    xi = num / den  # [cores, j4, g4, col]
    xi = xi.transpose(0, 2, 1, 3).reshape(BATCH)
    return np.ascontiguousarray(xi[:, None]).astype(np.float32)


def kernel(x, Wg, W1, b1, W2, b2):
    os.environ["BASS_NEVER_TRACE"] = "1"
    in_maps = make_in_maps(x, Wg, W1, b1, W2, b2)
    runner = get_runner(B_LOC)
    dev_in = runner.put_inputs(in_maps)
    outs = runner.run(dev_in)
    return decode_out(np.asarray(outs[0]))


if __name__ == "__main__":
    rng = np.random.default_rng(0)
    inputs = {
        "x": rng.standard_normal((BATCH, D_IN), dtype=np.float32),
        "Wg": (rng.standard_normal((N_EXPERTS, D_IN)) * 0.02).astype(np.float32),
        "W1": (rng.standard_normal((N_EXPERTS, D_IN, HID)) * 0.02).astype(np.float32),
        "b1": (rng.standard_normal((N_EXPERTS, HID)) * 0.02).astype(np.float32),
        "W2": (rng.standard_normal((N_EXPERTS, HID)) * 0.02).astype(np.float32),
        "b2": (rng.standard_normal((N_EXPERTS,)) * 0.02).astype(np.float32),
    }
    out = kernel(**inputs)
    print(out.shape, out.dtype, out[:4, 0])
